# revision 37
# baseline (speedup 1.0000x reference)
"""Causal Group-Query Attention kernel for Trainium2 (8 NeuronCores, SPMD).

Problem: x[2,2048,2048] @ Wq -> q(16 heads x 128); x @ Wkv -> k,v (4 KV heads);
causal softmax attention with GQA (4 q-heads per kv-head); y @ Wc -> out.

Sharding (2 batch x 4 head-groups = 8 cores):
  core = 4*b + g handles batch b, q-heads 4g..4g+3 (= kv head g).
  Each core gets xT (x[b] transposed, [C,T]), its Wq/Wk/Wv column shards and
  Wc row shard, and produces a partial [T,C] output; host sums the 4 partials
  per batch (the "all-reduce" of the c_proj happens on host).

Per-core device pipeline, software-pipelined over 512-wide t strips
(matmuls in f32r = fp32 HIGH mode, ~tf32 precision at near-bf16 rate):
  per strip s: projections (qT strip per head, kT strip, v strip via
  PE transpose); then per head: S^T blocks [tk=128, tq=512], exp on ScalarE
  (softmax scale fused), causal diagonal masks on DVE, denominator row via
  ones-column matmul accumulation, yT via matmul(lhsT=v_block, rhs=p_block),
  normalization (denom -> PE outer-product broadcast -> DVE reciprocal ->
  multiply); then c_proj for the strip's 4 t-tiles, DMA out.
All PSUM accumulators share one 4-slot pool; S^T pairs use a 2x2-bank pool.
"""

import sys

sys.path.insert(0, "/opt/trn_rl_repo")

import numpy as np

import concourse.bass as bass  # noqa: F401
import concourse.tile as tile
from concourse import bacc, mybir
from concourse.masks import make_identity

F32 = mybir.dt.float32
F32R = mybir.dt.float32r
BF16 = mybir.dt.bfloat16

T_FULL = 2048
C = 2048          # model dim (contraction for projections)
D = 128           # head dim
HPC = 4           # heads per core
P = 128
CI = C // P       # 16 contraction tiles
CG = 8            # ci-tiles per xt half-tile
SCALE = 1.0 / float(np.sqrt(D))


def build_nc(T=T_FULL):
    """Build and compile the per-core Bass module. T: multiple of 512."""
    assert T % 512 == 0
    TS = T // 512

    nc = bacc.Bacc("TRN2", target_bir_lowering=False, debug=False,
                   enable_asserts=True, num_devices=8)

    xt_d = nc.dram_tensor("xt", [C, T], BF16, kind="ExternalInput").ap()
    wq_d = nc.dram_tensor("wq", [C, HPC * D], BF16, kind="ExternalInput").ap()
    wk_d = nc.dram_tensor("wk", [C, D], BF16, kind="ExternalInput").ap()
    wv_d = nc.dram_tensor("wv", [C, D], BF16, kind="ExternalInput").ap()
    wc_d = nc.dram_tensor("wc", [HPC * D, C], BF16, kind="ExternalInput").ap()
    mask_d = nc.dram_tensor("mask", [4, P, 512], BF16, kind="ExternalInput").ap()
    ones_d = nc.dram_tensor("ones", [P, P], BF16, kind="ExternalInput").ap()
    out_d = nc.dram_tensor("out", [T, C], F32, kind="ExternalOutput").ap()

    xt_v = xt_d.rearrange("(ci p) t -> p ci t", p=P)          # [128,16,T]
    wq_v = wq_d.rearrange("(ci p) e -> p ci e", p=P)          # [128,16,512]
    wk_v = wk_d.rearrange("(ci p) d -> p ci d", p=P)          # [128,16,128]
    wv_v = wv_d.rearrange("(ci p) d -> p ci d", p=P)          # [128,16,128]
    wc_v = wc_d.rearrange("(hh p) o -> p hh o", p=P)          # [128,4,C]
    mask_v = mask_d.rearrange("b p c -> p b c")               # [128,4,512]
    out_v = out_d.rearrange("(tt p) (os o) -> p tt os o", p=P, o=512)

    with tile.TileContext(nc) as tc:
        with (
            tc.tile_pool(name="consts", bufs=1) as consts,
            tc.tile_pool(name="weights", bufs=1) as weights,
            tc.tile_pool(name="persist", bufs=1) as persist,
            tc.tile_pool(name="xtp", bufs=2) as xtp,
            tc.tile_pool(name="qtp", bufs=3) as qtp,
            tc.tile_pool(name="ytp", bufs=3) as ytp,
            tc.tile_pool(name="vts", bufs=1) as vts,
            tc.tile_pool(name="pp", bufs=6) as pp,
            tc.tile_pool(name="np_", bufs=3) as np_,
            tc.tile_pool(name="op", bufs=6) as op,
            tc.tile_pool(name="acc", bufs=4, space="PSUM") as acc,
            tc.tile_pool(name="sp", bufs=2, space="PSUM") as sp,
        ):
            # --- weights / consts; first strip's x chunks get priority ---
            # (emission order = DMA issue order: x(s0)+wq chunks first so the
            # first projection matmuls start ~6us in, not after all inputs)
            xt_sb0 = [xtp.tile([P, CG, 512], BF16, tag="xt", name=f"xt0_{i}")
                      for i in range(CI // CG)]
            wq_sb = weights.tile([P, CI, HPC * D], BF16, tag="wq")
            wk_sb = weights.tile([P, CI, D], BF16, tag="wk")
            wv_sb = weights.tile([P, CI, D], BF16, tag="wv")
            def _xtq0(q):
                nc.sync.dma_start(xt_sb0[q // 2][:, (q % 2) * 4:(q % 2) * 4 + 4, :],
                                  xt_v[:, q * 4:(q + 1) * 4, 0:512])
            _xtq0(0)
            nc.sync.dma_start(wk_sb[:], wk_v)
            nc.sync.dma_start(wv_sb[:], wv_v)
            _xtq0(1)
            nc.sync.dma_start(wq_sb[:, 0:4, :], wq_v[:, 0:4, :])
            nc.sync.dma_start(wq_sb[:, 4:8, :], wq_v[:, 4:8, :])
            _xtq0(2)
            nc.sync.dma_start(wq_sb[:, 8:12, :], wq_v[:, 8:12, :])
            _xtq0(3)
            nc.sync.dma_start(wq_sb[:, 12:16, :], wq_v[:, 12:16, :])
            mask_sb = consts.tile([P, 4, 512], BF16, tag="mask")
            nc.sync.dma_start(mask_sb[:], mask_v)
            ones_sb = consts.tile([P, P], BF16, tag="ones")
            nc.sync.dma_start(ones_sb[:], ones_d)
            ident = consts.tile([P, P], F32, tag="ident")
            make_identity(nc, ident[:])
            wc_sb = weights.tile([P, HPC, C], BF16, tag="wc")
            for cg in range(2):
                nc.sync.dma_start(wc_sb[:, :, cg * C // 2:(cg + 1) * C // 2],
                                  wc_v[:, :, cg * C // 2:(cg + 1) * C // 2])

            kt_sb = persist.tile([P, T], BF16, tag="kt")        # [d, t]
            v_sb = persist.tile([P, T // P, D], BF16, tag="v")  # [t, tt, d]

            xt_next = xt_sb0
            for s in range(TS):
                sl = slice(s * 512, (s + 1) * 512)
                xt_sb = xt_next

                # ---- projections for strip s ----
                # k and v interleaved per ci-quarter so the first matmuls
                # only need the first xT quarter-chunk (startup overlap)
                kps = acc.tile([P, 512], F32, tag="acc")         # kT strip
                vps = acc.tile([P, 512], F32, tag="acc")         # vT strip
                q0ps = acc.tile([P, 512], F32, tag="acc")        # qT head 0
                q1ps = acc.tile([P, 512], F32, tag="acc")        # qT head 1
                for cq in range(4):
                    for tgt, wsl in ((kps, wk_sb), (vps, wv_sb)):
                        for ci in range(cq * 4, cq * 4 + 4):
                            nc.tensor.matmul(
                                tgt[:], lhsT=wsl[:, ci, :],
                                rhs=xt_sb[ci // CG][:, ci % CG, :],
                                start=(ci == 0), stop=(ci == CI - 1))
                    for tgt, e in ((q0ps, 0), (q1ps, 1)):
                        for ci in range(cq * 4, cq * 4 + 4):
                            nc.tensor.matmul(
                                tgt[:], lhsT=wq_sb[:, ci, e * D:(e + 1) * D],
                                rhs=xt_sb[ci // CG][:, ci % CG, :],
                                start=(ci == 0), stop=(ci == CI - 1))
                nc.vector.tensor_copy(out=kt_sb[:, sl], in_=kps[:])
                qt_sb = qtp.tile([P, HPC, 512], BF16, tag="qt")  # [d, h, tq]
                nc.vector.tensor_copy(out=qt_sb[:, 0, :], in_=q0ps[:])
                nc.vector.tensor_copy(out=qt_sb[:, 1, :], in_=q1ps[:])
                vt_sb = vts.tile([P, 512], F32, tag="vt")
                nc.vector.tensor_copy(out=vt_sb[:], in_=vps[:])
                for k in range(4):    # PE transpose -> v natural [t, d]
                    tp = acc.tile([P, P], F32, tag="acc")
                    nc.tensor.transpose(tp[:], vt_sb[:, k * P:(k + 1) * P],
                                        ident[:])
                    nc.vector.tensor_copy(out=v_sb[:, s * 4 + k, :], in_=tp[:])

                for e in range(2, HPC):
                    ps = acc.tile([P, 512], F32, tag="acc")
                    for ci in range(CI):
                        nc.tensor.matmul(
                            ps[:], lhsT=wq_sb[:, ci, e * D:(e + 1) * D],
                            rhs=xt_sb[ci // CG][:, ci % CG, :],
                            start=(ci == 0), stop=(ci == CI - 1))
                    nc.vector.tensor_copy(out=qt_sb[:, e, :], in_=ps[:])

                # prefetch next strip's xT while attention runs
                if s + 1 < TS:
                    xt_next = [xtp.tile([P, CG, 512], BF16, tag="xt",
                                        name=f"xt{s + 1}_{i}")
                               for i in range(CI // CG)]
                    nsl = slice((s + 1) * 512, (s + 2) * 512)
                    for q in range(4):
                        nc.sync.dma_start(
                            xt_next[q // 2][:, (q % 2) * 4:(q % 2) * 4 + 4, :],
                            xt_v[:, q * 4:(q + 1) * 4, nsl])

                # ---- attention for strip s, all heads ----
                # Software skew carried ACROSS heads: PV/dn run a few items
                # behind S/exp so the exp+mask chain never stalls the PE
                # stream. Full (off-diagonal) tk blocks go in pairs; the 4
                # diagonal blocks go as singles at offset 128*b (block b only
                # covers tq >= 128b; on the shifted range every diagonal
                # block's causal mask is pattern 0).
                yt_sb = ytp.tile([P, HPC, 512], BF16, tag="yt")  # [d, h, tq]
                nblk = 4 * s + 4          # causal: tk tiles j = 0..nblk-1
                pv_q = []

                def emit_pv(p_sb, specs, yt_ps, dn_ps, h):
                    for u, j, off, n in specs:
                        nc.tensor.matmul(
                            yt_ps[:, off:], lhsT=v_sb[:, j, :],
                            rhs=p_sb[:, u, :n],
                            start=(j == 0), stop=(j == nblk - 1))
                        nc.tensor.matmul(
                            dn_ps[:, off:], lhsT=ones_sb[:],
                            rhs=p_sb[:, u, :n],
                            start=(j == 0), stop=(j == nblk - 1))
                    if specs[-1][1] == nblk - 1:   # head complete: normalize
                        drecip = np_.tile([P, 512], F32, tag="drecip")
                        nc.vector.reciprocal_approx_fast(
                            out=drecip[:], in_=dn_ps[:])
                        nc.vector.tensor_mul(
                            out=yt_sb[:, h, :], in0=yt_ps[:], in1=drecip[:])

                for h in range(HPC):
                    yt_ps = acc.tile([P, 512], F32, tag="acc")
                    dn_ps = acc.tile([P, 512], F32, tag="acc")
                    # full blocks in pairs
                    for jp in range(0, 4 * s, 2):
                        s_ps = sp.tile([P, 2, 512], F32, tag="s_ps")
                        for u in range(2):
                            j = jp + u
                            nc.tensor.matmul(
                                s_ps[:, u, :],
                                lhsT=kt_sb[:, j * P:(j + 1) * P],
                                rhs=qt_sb[:, h, :],
                                start=True, stop=True)
                        p_sb = pp.tile([P, 2, 512], BF16, tag="p_sb")
                        nc.scalar.activation(
                            p_sb[:], s_ps[:],
                            mybir.ActivationFunctionType.Exp, scale=SCALE)
                        pv_q.append((p_sb, [(0, jp, 0, 512), (1, jp + 1, 0, 512)],
                                     yt_ps, dn_ps, h))
                        if len(pv_q) > 4:
                            emit_pv(*pv_q.pop(0))
                    # diagonal blocks as singles at offset 128*b
                    for b2 in range(0, 4, 2):
                        s_ps = sp.tile([P, 2, 512], F32, tag="s_ps")
                        specs2 = []
                        for u in range(2):
                            b = b2 + u
                            j = 4 * s + b
                            off = 128 * b
                            n = 512 - off
                            nc.tensor.matmul(
                                s_ps[:, u, :n],
                                lhsT=kt_sb[:, j * P:(j + 1) * P],
                                rhs=qt_sb[:, h, off:],
                                start=True, stop=True)
                            specs2.append((u, j, off, n))
                        p_sb = pp.tile([P, 2, 512], BF16, tag="p_sb")
                        for u, j, off, n in specs2:
                            nc.scalar.activation(
                                p_sb[:, u, :n], s_ps[:, u, :n],
                                mybir.ActivationFunctionType.Exp, scale=SCALE)
                            nc.vector.tensor_mul(
                                out=p_sb[:, u, :n], in0=p_sb[:, u, :n],
                                in1=mask_sb[:, 0, :n])
                        pv_q.append((p_sb, specs2, yt_ps, dn_ps, h))
                        if len(pv_q) > 4:
                            emit_pv(*pv_q.pop(0))
                for item in pv_q:
                    emit_pv(*item)
                pv_q = []

                # ---- c_proj for strip s (t tiles 4s..4s+3) ----
                for tr in range(4):
                    tt = 4 * s + tr
                    for os_ in range(4):
                        ps = acc.tile([P, 512], F32, tag="acc")
                        for hh in range(HPC):
                            nc.tensor.matmul(
                                ps[:],
                                lhsT=yt_sb[:, hh, tr * P:(tr + 1) * P],
                                rhs=wc_sb[:, hh, os_ * 512:(os_ + 1) * 512],
                                start=(hh == 0), stop=(hh == HPC - 1))
                        o_sb = op.tile([P, 512], F32, tag="out_sb")
                        nc.scalar.copy(out=o_sb[:], in_=ps[:])
                        nc.gpsimd.dma_start(out_v[:, tt, os_], o_sb[:])

    nc.compile()
    return nc


def make_masks():
    r = np.arange(P)[:, None]
    c = np.arange(512)[None, :]
    return np.ascontiguousarray(
        np.stack([(c >= 128 * b + r) for b in range(4)]).astype(np.float32))


def make_in_maps(x, Wq, Wkv, Wc):
    import ml_dtypes
    bf16 = ml_dtypes.bfloat16
    masks = make_masks().astype(bf16)
    in_maps = []
    for core in range(8):
        b, g = core // 4, core % 4
        in_maps.append({
            "xt": np.ascontiguousarray(np.asarray(x[b]).T).astype(bf16),
            "wq": np.ascontiguousarray(
                np.asarray(Wq[:, 512 * g:512 * (g + 1)])).astype(bf16),
            "wk": np.ascontiguousarray(
                np.asarray(Wkv[:, 128 * g:128 * (g + 1)])).astype(bf16),
            "wv": np.ascontiguousarray(
                np.asarray(Wkv[:, 512 + 128 * g:512 + 128 * (g + 1)])).astype(bf16),
            "wc": np.ascontiguousarray(
                np.asarray(Wc[512 * g:512 * (g + 1), :])).astype(bf16),
            "mask": masks,
            "ones": np.ones((P, P), bf16),
        })
    return in_maps


_NC_CACHE = {}


def _get_nc():
    if "nc" not in _NC_CACHE:
        _NC_CACHE["nc"] = build_nc()
    return _NC_CACHE["nc"]


def run(x, Wq, Wkv, Wc, trace=False, **kwargs):
    from concourse.bass_utils import run_bass_kernel_spmd
    nc = _get_nc()
    in_maps = make_in_maps(x, Wq, Wkv, Wc)
    res = run_bass_kernel_spmd(nc, in_maps, list(range(8)), trace=trace, **kwargs)
    B, T, C_ = x.shape
    out = np.empty((B, T, C_), np.float32)
    for b in range(B):
        acc = res.results[4 * b]["out"].astype(np.float32)
        for g in range(1, 4):
            acc = acc + res.results[4 * b + g]["out"]
        out[b] = acc
    return out, res


def kernel(x, Wq, Wkv, Wc):
    out, _ = run(x, Wq, Wkv, Wc, trace=False)
    return out


# revision 39
# speedup vs baseline: 1.0184x; 1.0184x over previous
"""Causal Group-Query Attention kernel for Trainium2 (8 NeuronCores, SPMD).

Problem: x[2,2048,2048] @ Wq -> q(16 heads x 128); x @ Wkv -> k,v (4 KV heads);
causal softmax attention with GQA (4 q-heads per kv-head); y @ Wc -> out.

Sharding (2 batch x 4 head-groups = 8 cores):
  core = 4*b + g handles batch b, q-heads 4g..4g+3 (= kv head g).
  Each core gets xT (x[b] transposed, [C,T]), its Wq/Wk/Wv column shards and
  Wc row shard, and produces a partial [T,C] output; host sums the 4 partials
  per batch (the "all-reduce" of the c_proj happens on host).

Per-core device pipeline, software-pipelined over 512-wide t strips
(matmuls in f32r = fp32 HIGH mode, ~tf32 precision at near-bf16 rate):
  per strip s: projections (qT strip per head, kT strip, v strip via
  PE transpose); then per head: S^T blocks [tk=128, tq=512], exp on ScalarE
  (softmax scale fused), causal diagonal masks on DVE, denominator row via
  ones-column matmul accumulation, yT via matmul(lhsT=v_block, rhs=p_block),
  normalization (denom -> PE outer-product broadcast -> DVE reciprocal ->
  multiply); then c_proj for the strip's 4 t-tiles, DMA out.
All PSUM accumulators share one 4-slot pool; S^T pairs use a 2x2-bank pool.
"""

import sys

sys.path.insert(0, "/opt/trn_rl_repo")

import numpy as np

import concourse.bass as bass  # noqa: F401
import concourse.tile as tile
from concourse import bacc, mybir
from concourse.masks import make_identity

F32 = mybir.dt.float32
F32R = mybir.dt.float32r
BF16 = mybir.dt.bfloat16

T_FULL = 2048
C = 2048          # model dim (contraction for projections)
D = 128           # head dim
HPC = 4           # heads per core
P = 128
CI = C // P       # 16 contraction tiles
CG = 8            # ci-tiles per xt half-tile
SCALE = 1.0 / float(np.sqrt(D))


def build_nc(T=T_FULL):
    """Build and compile the per-core Bass module. T: multiple of 512."""
    assert T % 512 == 0
    TS = T // 512

    nc = bacc.Bacc("TRN2", target_bir_lowering=False, debug=False,
                   enable_asserts=True, num_devices=8)

    xt_d = nc.dram_tensor("xt", [C, T], BF16, kind="ExternalInput").ap()
    wq_d = nc.dram_tensor("wq", [C, HPC * D], BF16, kind="ExternalInput").ap()
    wk_d = nc.dram_tensor("wk", [C, D], BF16, kind="ExternalInput").ap()
    wv_d = nc.dram_tensor("wv", [C, D], BF16, kind="ExternalInput").ap()
    wc_d = nc.dram_tensor("wc", [HPC * D, C], BF16, kind="ExternalInput").ap()
    mask_d = nc.dram_tensor("mask", [4, P, 512], BF16, kind="ExternalInput").ap()
    ones_d = nc.dram_tensor("ones", [P, P], BF16, kind="ExternalInput").ap()
    out_d = nc.dram_tensor("out", [T, C], F32, kind="ExternalOutput").ap()

    xt_v = xt_d.rearrange("(ci p) t -> p ci t", p=P)          # [128,16,T]
    wq_v = wq_d.rearrange("(ci p) e -> p ci e", p=P)          # [128,16,512]
    wk_v = wk_d.rearrange("(ci p) d -> p ci d", p=P)          # [128,16,128]
    wv_v = wv_d.rearrange("(ci p) d -> p ci d", p=P)          # [128,16,128]
    wc_v = wc_d.rearrange("(hh p) o -> p hh o", p=P)          # [128,4,C]
    mask_v = mask_d.rearrange("b p c -> p b c")               # [128,4,512]
    out_v = out_d.rearrange("(tt p) (os o) -> p tt os o", p=P, o=512)

    with tile.TileContext(nc) as tc:
        with (
            tc.tile_pool(name="consts", bufs=1) as consts,
            tc.tile_pool(name="weights", bufs=1) as weights,
            tc.tile_pool(name="persist", bufs=1) as persist,
            tc.tile_pool(name="xtp", bufs=2) as xtp,
            tc.tile_pool(name="qtp", bufs=2) as qtp,
            tc.tile_pool(name="ytp", bufs=2) as ytp,
            tc.tile_pool(name="vts", bufs=1) as vts,
            tc.tile_pool(name="pp", bufs=7) as pp,
            tc.tile_pool(name="np_", bufs=3) as np_,
            tc.tile_pool(name="op", bufs=6) as op,
            tc.tile_pool(name="acc", bufs=4, space="PSUM") as acc,
            tc.tile_pool(name="sp", bufs=2, space="PSUM") as sp,
        ):
            # --- weights / consts; first strip's x chunks get priority ---
            # (emission order = DMA issue order: x(s0)+wq chunks first so the
            # first projection matmuls start ~6us in, not after all inputs)
            xt_sb0 = [xtp.tile([P, CG, 512], BF16, tag="xt", name=f"xt0_{i}")
                      for i in range(CI // CG)]
            wq_sb = weights.tile([P, CI, HPC * D], BF16, tag="wq")
            wk_sb = weights.tile([P, CI, D], BF16, tag="wk")
            wv_sb = weights.tile([P, CI, D], BF16, tag="wv")
            def _xtq0(q):
                nc.sync.dma_start(xt_sb0[q // 2][:, (q % 2) * 4:(q % 2) * 4 + 4, :],
                                  xt_v[:, q * 4:(q + 1) * 4, 0:512])
            _xtq0(0)
            nc.sync.dma_start(wk_sb[:], wk_v)
            nc.sync.dma_start(wv_sb[:], wv_v)
            _xtq0(1)
            nc.sync.dma_start(wq_sb[:, 0:4, :], wq_v[:, 0:4, :])
            nc.sync.dma_start(wq_sb[:, 4:8, :], wq_v[:, 4:8, :])
            _xtq0(2)
            nc.sync.dma_start(wq_sb[:, 8:12, :], wq_v[:, 8:12, :])
            _xtq0(3)
            nc.sync.dma_start(wq_sb[:, 12:16, :], wq_v[:, 12:16, :])
            mask_sb = consts.tile([P, 4, 512], BF16, tag="mask")
            nc.sync.dma_start(mask_sb[:], mask_v)
            ones_sb = consts.tile([P, P], BF16, tag="ones")
            nc.sync.dma_start(ones_sb[:], ones_d)
            ident = consts.tile([P, P], F32, tag="ident")
            make_identity(nc, ident[:])
            wc_sb = weights.tile([P, HPC, C], BF16, tag="wc")
            for cg in range(2):
                nc.sync.dma_start(wc_sb[:, :, cg * C // 2:(cg + 1) * C // 2],
                                  wc_v[:, :, cg * C // 2:(cg + 1) * C // 2])

            kt_sb = persist.tile([P, T], BF16, tag="kt")        # [d, t]
            v_sb = persist.tile([P, T // P, D], BF16, tag="v")  # [t, tt, d]

            xt_next = xt_sb0
            for s in range(TS):
                sl = slice(s * 512, (s + 1) * 512)
                xt_sb = xt_next

                # ---- projections for strip s ----
                # k and v interleaved per ci-quarter so the first matmuls
                # only need the first xT quarter-chunk (startup overlap)
                kps = acc.tile([P, 512], F32, tag="acc")         # kT strip
                vps = acc.tile([P, 512], F32, tag="acc")         # vT strip
                q0ps = acc.tile([P, 512], F32, tag="acc")        # qT head 0
                q1ps = acc.tile([P, 512], F32, tag="acc")        # qT head 1
                for cq in range(4):
                    for tgt, wsl in ((kps, wk_sb), (vps, wv_sb)):
                        for ci in range(cq * 4, cq * 4 + 4):
                            nc.tensor.matmul(
                                tgt[:], lhsT=wsl[:, ci, :],
                                rhs=xt_sb[ci // CG][:, ci % CG, :],
                                start=(ci == 0), stop=(ci == CI - 1))
                    for tgt, e in ((q0ps, 0), (q1ps, 1)):
                        for ci in range(cq * 4, cq * 4 + 4):
                            nc.tensor.matmul(
                                tgt[:], lhsT=wq_sb[:, ci, e * D:(e + 1) * D],
                                rhs=xt_sb[ci // CG][:, ci % CG, :],
                                start=(ci == 0), stop=(ci == CI - 1))
                nc.vector.tensor_copy(out=kt_sb[:, sl], in_=kps[:])
                qt_sb = qtp.tile([P, HPC, 512], BF16, tag="qt")  # [d, h, tq]
                nc.vector.tensor_copy(out=qt_sb[:, 0, :], in_=q0ps[:])
                nc.vector.tensor_copy(out=qt_sb[:, 1, :], in_=q1ps[:])
                vt_sb = vts.tile([P, 512], F32, tag="vt")
                nc.vector.tensor_copy(out=vt_sb[:], in_=vps[:])
                for k in range(4):    # PE transpose -> v natural [t, d]
                    tp = acc.tile([P, P], F32, tag="acc")
                    nc.tensor.transpose(tp[:], vt_sb[:, k * P:(k + 1) * P],
                                        ident[:])
                    nc.vector.tensor_copy(out=v_sb[:, s * 4 + k, :], in_=tp[:])

                for e in range(2, HPC):
                    ps = acc.tile([P, 512], F32, tag="acc")
                    for ci in range(CI):
                        nc.tensor.matmul(
                            ps[:], lhsT=wq_sb[:, ci, e * D:(e + 1) * D],
                            rhs=xt_sb[ci // CG][:, ci % CG, :],
                            start=(ci == 0), stop=(ci == CI - 1))
                    nc.vector.tensor_copy(out=qt_sb[:, e, :], in_=ps[:])

                # prefetch next strip's xT while attention runs
                if s + 1 < TS:
                    xt_next = [xtp.tile([P, CG, 512], BF16, tag="xt",
                                        name=f"xt{s + 1}_{i}")
                               for i in range(CI // CG)]
                    nsl = slice((s + 1) * 512, (s + 2) * 512)
                    for q in range(4):
                        nc.sync.dma_start(
                            xt_next[q // 2][:, (q % 2) * 4:(q % 2) * 4 + 4, :],
                            xt_v[:, q * 4:(q + 1) * 4, nsl])

                # ---- attention for strip s, all heads ----
                # Software skew carried ACROSS heads: PV/dn run a few items
                # behind S/exp so the exp+mask chain never stalls the PE
                # stream. Full (off-diagonal) tk blocks go in pairs; the 4
                # diagonal blocks go as singles at offset 128*b (block b only
                # covers tq >= 128b; on the shifted range every diagonal
                # block's causal mask is pattern 0).
                yt_sb = ytp.tile([P, HPC, 512], BF16, tag="yt")  # [d, h, tq]
                nblk = 4 * s + 4          # causal: tk tiles j = 0..nblk-1
                pv_q = []

                def emit_pv(p_sb, specs, yt_ps, dn_ps, h):
                    for u, j, off, n in specs:
                        nc.tensor.matmul(
                            yt_ps[:, off:], lhsT=v_sb[:, j, :],
                            rhs=p_sb[:, u, :n],
                            start=(j == 0), stop=(j == nblk - 1))
                        nc.tensor.matmul(
                            dn_ps[:, off:], lhsT=ones_sb[:],
                            rhs=p_sb[:, u, :n],
                            start=(j == 0), stop=(j == nblk - 1))
                    if specs[-1][1] == nblk - 1:   # head complete: normalize
                        drecip = np_.tile([P, 512], F32, tag="drecip")
                        nc.vector.reciprocal_approx_fast(
                            out=drecip[:], in_=dn_ps[:])
                        nc.vector.tensor_mul(
                            out=yt_sb[:, h, :], in0=yt_ps[:], in1=drecip[:])

                for h in range(HPC):
                    yt_ps = acc.tile([P, 512], F32, tag="acc")
                    dn_ps = acc.tile([P, 512], F32, tag="acc")
                    # full blocks in pairs
                    for jp in range(0, 4 * s, 2):
                        s_ps = sp.tile([P, 2, 512], F32, tag="s_ps")
                        for u in range(2):
                            j = jp + u
                            nc.tensor.matmul(
                                s_ps[:, u, :],
                                lhsT=kt_sb[:, j * P:(j + 1) * P],
                                rhs=qt_sb[:, h, :],
                                start=True, stop=True)
                        p_sb = pp.tile([P, 2, 512], BF16, tag="p_sb")
                        nc.scalar.activation(
                            p_sb[:], s_ps[:],
                            mybir.ActivationFunctionType.Exp, scale=SCALE)
                        pv_q.append((p_sb, [(0, jp, 0, 512), (1, jp + 1, 0, 512)],
                                     yt_ps, dn_ps, h))
                        if len(pv_q) > 5:
                            emit_pv(*pv_q.pop(0))
                    # diagonal blocks as singles at offset 128*b
                    for b2 in range(0, 4, 2):
                        s_ps = sp.tile([P, 2, 512], F32, tag="s_ps")
                        specs2 = []
                        for u in range(2):
                            b = b2 + u
                            j = 4 * s + b
                            off = 128 * b
                            n = 512 - off
                            nc.tensor.matmul(
                                s_ps[:, u, :n],
                                lhsT=kt_sb[:, j * P:(j + 1) * P],
                                rhs=qt_sb[:, h, off:],
                                start=True, stop=True)
                            specs2.append((u, j, off, n))
                        p_sb = pp.tile([P, 2, 512], BF16, tag="p_sb")
                        for u, j, off, n in specs2:
                            nc.scalar.activation(
                                p_sb[:, u, :n], s_ps[:, u, :n],
                                mybir.ActivationFunctionType.Exp, scale=SCALE)
                            nc.vector.tensor_mul(
                                out=p_sb[:, u, :n], in0=p_sb[:, u, :n],
                                in1=mask_sb[:, 0, :n])
                        pv_q.append((p_sb, specs2, yt_ps, dn_ps, h))
                        if len(pv_q) > 5:
                            emit_pv(*pv_q.pop(0))
                for item in pv_q:
                    emit_pv(*item)
                pv_q = []

                # ---- c_proj for strip s (t tiles 4s..4s+3) ----
                for tr in range(4):
                    tt = 4 * s + tr
                    for os_ in range(4):
                        ps = acc.tile([P, 512], F32, tag="acc")
                        for hh in range(HPC):
                            nc.tensor.matmul(
                                ps[:],
                                lhsT=yt_sb[:, hh, tr * P:(tr + 1) * P],
                                rhs=wc_sb[:, hh, os_ * 512:(os_ + 1) * 512],
                                start=(hh == 0), stop=(hh == HPC - 1))
                        o_sb = op.tile([P, 512], F32, tag="out_sb")
                        nc.scalar.copy(out=o_sb[:], in_=ps[:])
                        nc.gpsimd.dma_start(out_v[:, tt, os_], o_sb[:])

    nc.compile()
    return nc


def make_masks():
    r = np.arange(P)[:, None]
    c = np.arange(512)[None, :]
    return np.ascontiguousarray(
        np.stack([(c >= 128 * b + r) for b in range(4)]).astype(np.float32))


def make_in_maps(x, Wq, Wkv, Wc):
    import ml_dtypes
    bf16 = ml_dtypes.bfloat16
    masks = make_masks().astype(bf16)
    in_maps = []
    for core in range(8):
        b, g = core // 4, core % 4
        in_maps.append({
            "xt": np.ascontiguousarray(np.asarray(x[b]).T).astype(bf16),
            "wq": np.ascontiguousarray(
                np.asarray(Wq[:, 512 * g:512 * (g + 1)])).astype(bf16),
            "wk": np.ascontiguousarray(
                np.asarray(Wkv[:, 128 * g:128 * (g + 1)])).astype(bf16),
            "wv": np.ascontiguousarray(
                np.asarray(Wkv[:, 512 + 128 * g:512 + 128 * (g + 1)])).astype(bf16),
            "wc": np.ascontiguousarray(
                np.asarray(Wc[512 * g:512 * (g + 1), :])).astype(bf16),
            "mask": masks,
            "ones": np.ones((P, P), bf16),
        })
    return in_maps


_NC_CACHE = {}


def _get_nc():
    if "nc" not in _NC_CACHE:
        _NC_CACHE["nc"] = build_nc()
    return _NC_CACHE["nc"]


def run(x, Wq, Wkv, Wc, trace=False, **kwargs):
    from concourse.bass_utils import run_bass_kernel_spmd
    nc = _get_nc()
    in_maps = make_in_maps(x, Wq, Wkv, Wc)
    res = run_bass_kernel_spmd(nc, in_maps, list(range(8)), trace=trace, **kwargs)
    B, T, C_ = x.shape
    out = np.empty((B, T, C_), np.float32)
    for b in range(B):
        acc = res.results[4 * b]["out"].astype(np.float32)
        for g in range(1, 4):
            acc = acc + res.results[4 * b + g]["out"]
        out[b] = acc
    return out, res


def kernel(x, Wq, Wkv, Wc):
    out, _ = run(x, Wq, Wkv, Wc, trace=False)
    return out
